# revision 78
# baseline (speedup 1.0000x reference)
# Trainium2 Bass kernel for nn_Attention_57509612094021 (XCA-style channel attention).
#
# Sharding: pure data-parallel over batch (8 images -> 8 NeuronCores), no collectives.
# Per-core pipeline (one [192,128,128] image):
#   - qk path all-fp8: 1x1 conv as fp8 DoubleRow matmuls (3 slabs of 128),
#     PSUM -> fp8 zero-padded SBUF slabs, depthwise 3x3 on PE as fp8
#     DoubleRow diag-matmuls (4 tap-pairs + center), PSUM -> fp16 qk_blk,
#   - v path fp16 (v feeds the output linearly, fp8 fails the error budget):
#     1x1 conv fp16 (2-pass contraction 128+64) -> fp16 padded slabs;
#     depthwise 3x3 WITHOUT PSUM: accumulated in-place into v_buf by DVE
#     (tensor_scalar mult + tensor_tensor add pairs) and the otherwise-idle
#     Pool/GpSimd engine (fused scalar_tensor_tensor mult-add), chains
#     anti-phased across the two slabs so both engines stay busy,
#   - PSUM->SBUF copies greedily balanced across ACT/DVE by a load model,
#   - per-block batched DMA-xbar transposes of q|k -> stacked per-head [96,96]
#     Gram accumulated in PSUM across the whole image (diag = L2 norms),
#   - rsqrt + Newton step, temperature/row/col scaling, softmax (exp on ACT),
#     proj FUSED into attention: M_h = A_h @ Wp_h, y = sum_h M_h^T.T @ v_h.
# Host side pre-permutes weights (head-interleaved qk, 64-aligned v slots) and
# builds the fp8 diagonal depthwise matrices, masks, and selector matrices.
import os
import sys
import time

sys.path.insert(0, "/opt/trn_rl_repo")
os.environ.setdefault("JAX_PLATFORMS", "axon")

import numpy as np
import ml_dtypes

import concourse.bass as bass
import concourse.tile as tile
from concourse import bacc, mybir
from concourse.bass_utils import run_bass_kernel_spmd

# Prefer the act-table set that covers BOTH Ln and Exp: the load-insertion
# pass picks the first covering set per func, and with the default tables it
# alternates exp_and_others / natural_log_exp_and_others (4 swaps). Blank out
# exp_and_others (set ids are positional, so order must be preserved) so Exp
# resolves to natural_log_exp_and_others and the kernel lives in one set.
_orig_get_activation_tables = bacc.get_activation_tables

def _get_activation_tables_ln_exp_combined(arch):
    t = dict(_orig_get_activation_tables(arch))
    t["exp_and_others"] = set()
    return t

bacc.get_activation_tables = _get_activation_tables_ln_exp_combined

F32 = mybir.dt.float32
F32R = mybir.dt.float32r
BF16 = mybir.dt.bfloat16
F16 = mybir.dt.float16
FP8 = mybir.dt.float8e4
AF = mybir.ActivationFunctionType
OP = mybir.AluOpType
bf16 = ml_dtypes.bfloat16
fp16 = np.float16
fp8 = ml_dtypes.float8_e4m3

C = 192
O = 384          # 2C (qk out channels)
VSLOTS = 256     # v out slots (64-aligned heads, 192 live)
H = W = 128
HW = H * W
HEADS = 4
CPH = 48
R = 16           # rows per block
NB = H // R      # 8 blocks
PXB = R * W      # 2048 pixels per block
TAPS = [(ky, kx) for ky in range(3) for kx in range(3)]
DW_PAIRS = [(0, 2), (3, 5), (6, 8), (1, 7)]  # DoubleRow tap pairs; center 4 alone

# v depthwise tap split per slab: PE (fp16 diag matmuls, seeds v_buf via the
# PSUM copy) + DVE (tensor_scalar mult + tensor_tensor add pairs).
V_PE_TAPS = [[0, 1, 2, 3], [0, 1, 2, 3]]


def build_nc():
    nc = bacc.Bacc("TRN2", target_bir_lowering=False, debug=False, num_devices=8)

    d_x = nc.dram_tensor("x", [C, HW], F16, kind="ExternalInput").ap()
    d_x8 = nc.dram_tensor("x8", [96, 2 * HW], FP8, kind="ExternalInput").ap()
    d_wqk8 = nc.dram_tensor("wqk8", [96, 2 * O], FP8, kind="ExternalInput").ap()
    d_wvT = nc.dram_tensor("wvT", [C, VSLOTS], F16, kind="ExternalInput").ap()
    d_wphead = nc.dram_tensor("wphead", [48, 4 * C], F16, kind="ExternalInput").ap()
    d_diag8 = nc.dram_tensor("diag8", [128, 3 * 10 * 128], FP8, kind="ExternalInput").ap()
    d_diagv0 = nc.dram_tensor("diagv0", [128, 9 * 128], F16, kind="ExternalInput").ap()
    d_diagv1 = nc.dram_tensor("diagv1", [128, 9 * 128], F16, kind="ExternalInput").ap()
    d_dwv_sc = nc.dram_tensor("dwv_sc", [128, 18], F32, kind="ExternalInput").ap()
    d_gmask = nc.dram_tensor("gmask", [96, 4 * 96], F32, kind="ExternalInput").ap()
    d_tmmask = nc.dram_tensor("tmmask", [96, 4], F32, kind="ExternalInput").ap()
    d_idf32 = nc.dram_tensor("idf32", [128, 128], F32, kind="ExternalInput").ap()
    d_hsel = nc.dram_tensor("hsel", [4, 4 * 48], F32, kind="ExternalInput").ap()
    d_y = nc.dram_tensor("y", [C, HW], F16, kind="ExternalOutput").ap()

    with tile.TileContext(nc) as tc:
        # ---------------- pools ----------------
        consts = tc.alloc_tile_pool(name="consts", bufs=1)
        persist = tc.alloc_tile_pool(name="persist", bufs=1)
        xpool = tc.alloc_tile_pool(name="xpool", bufs=2)
        blkpool = tc.alloc_tile_pool(name="blkpool", bufs=2)
        qkbpool = tc.alloc_tile_pool(name="qkbpool", bufs=2)
        scpool = tc.alloc_tile_pool(name="scpool", bufs=1)
        pb_pool = tc.alloc_tile_pool(name="pbpool", bufs=1)
        ys_pool = tc.alloc_tile_pool(name="yspool", bufs=2)
        ps_gram = tc.alloc_tile_pool(name="ps_gram", bufs=1, space="PSUM")
        ps_conv = tc.alloc_tile_pool(name="ps_conv", bufs=2, space="PSUM")  # 2-bank tiles
        ps_dw = tc.alloc_tile_pool(name="ps_dw", bufs=3, space="PSUM")

        # ---------------- constants / weights ----------------
        wqk8 = consts.tile([96, 2, O], FP8, tag="wqk8")
        wvT_a = consts.tile([128, VSLOTS], F16, tag="wvT_a")
        wvT_b = consts.tile([64, VSLOTS], F16, tag="wvT_b")
        wphead = consts.tile([48, 4, C], F16, tag="wphead")
        diag8 = [consts.tile([128, 10, 128], FP8, tag=f"diag8_{s}", name=f"diag8_{s}")
                 for s in range(3)]
        diagv = [consts.tile([128, 9, 128], F16, tag=f"diagv{s}", name=f"diagv{s}")
                 for s in range(2)]
        dwv_sc = [consts.tile([128, 9], F32, tag=f"dwvsc{s}", name=f"dwvsc{s}") for s in range(2)]
        gmask = consts.tile([96, 4 * 96], F32, tag="gmask")
        tmmask = consts.tile([96, 4], F32, tag="tmmask")
        idf32 = consts.tile([128, 128], F32, tag="idf32")
        hsel = consts.tile([4, 4 * 48], F32, tag="hsel")

        # conv-critical consts first; the rest are deferred behind block 0's
        # input DMAs so the PE can start ~8us earlier.
        nc.sync.dma_start(wqk8[:], d_wqk8[:].rearrange("p (two o) -> p two o", two=2))

        def deferred_const_dmas():
            nc.sync.dma_start(wvT_a[:], d_wvT[0:128, :])
            nc.sync.dma_start(wvT_b[:], d_wvT[128:192, :])
            for s in range(3):
                nc.sync.dma_start(
                    diag8[s][:],
                    d_diag8[:, 1280 * s : 1280 * (s + 1)].rearrange("p (t c) -> p t c", t=10))
            nc.sync.dma_start(diagv[0][:], d_diagv0[:].rearrange("p (t c) -> p t c", t=9))
            nc.sync.dma_start(diagv[1][:], d_diagv1[:].rearrange("p (t c) -> p t c", t=9))
            for s in range(2):
                nc.sync.dma_start(dwv_sc[s][:], d_dwv_sc[:, 9 * s : 9 * (s + 1)])
            nc.sync.dma_start(wphead[:], d_wphead[:].rearrange("p (h o) -> p h o", h=4))
            nc.sync.dma_start(gmask[:], d_gmask[:])
            nc.sync.dma_start(tmmask[:], d_tmmask[:])
            nc.sync.dma_start(idf32[:], d_idf32[:])
            nc.sync.dma_start(hsel[:], d_hsel[:])

        # Preload the ONE act table set phase B needs: natural_log_exp covers
        # both Ln and Exp (rsqrt is computed as exp(-0.5*ln)), so no
        # LoadActFuncSet ever lands on the phase-B critical path.
        actwarm = consts.tile([1, 8], F32, tag="actwarm")
        nc.vector.memset(actwarm[:], 1.0)
        nc.scalar.activation(actwarm[:], actwarm[:], AF.Ln)
        nc.scalar.activation(actwarm[:], actwarm[:], AF.Exp)

        # ---------------- persistent tensors ----------------
        # v_buf split into per-block tiles so the dependency tracker lets the
        # out phase start on early blocks while late-block taps still run
        v_buf = [
            [persist.tile([128, PXB], F16, tag=f"v_buf{s}_{bb}", name=f"v_buf{s}_{bb}")
             for bb in range(NB)]
            for s in range(2)
        ]
        gram = ps_gram.tile([96, 4 * 96], F32, tag="gram")

        # padded conv-output slabs, manual parity double-buffer
        PADW = 132
        NPR = R + 2
        padq = [[persist.tile([128, NPR, PADW], FP8, tag=f"pq{s}{p}", name=f"pq{s}{p}")
                 for p in range(2)] for s in range(3)]
        padv = [[persist.tile([128, NPR, PADW], F16, tag=f"pv{s}{p}", name=f"pv{s}{p}")
                 for p in range(2)] for s in range(2)]
        # zero the side columns once (cols 0,1,130,131 never written later)
        for grp in (padq, padv):
            for row in grp:
                for t in row:
                    nc.gpsimd.memset(t[:, :, 0:2], 0.0)
                    nc.gpsimd.memset(t[:, :, 130:132], 0.0)
        # zero halo rows used by first block (parity 0)
        for grp in (padq, padv):
            for row in grp:
                nc.gpsimd.memset(row[0][:, 0:1, :], 0.0)

        # DVE scratch for v-depthwise tap products (one buffer: DVE is a
        # serial FIFO engine, ping-pong would buy nothing)
        dwsc = scpool.tile([128, PXB], F16, tag="dwsc")
        # Pool-engine scratch (half-block sized) for its broadcast-mult taps
        plsc = scpool.tile([128, PXB // 2], F16, tag="plsc")

        # ---------- balanced copy emitter (ACT / DVE; Pool can't read PSUM) ----------
        eng_load = {"act": 0.0, "dve": 0.0}
        RATE = {"act": 0.833, "dve": 1.042}
        OVH = {"act": 150.0, "dve": 130.0}

        def emit_copy(dst, src, elems, force=None):
            if force is not None:
                e = force
            else:
                e = min(eng_load, key=lambda k: eng_load[k] + RATE[k] * elems + OVH[k])
            eng_load[e] += RATE[e] * elems + OVH[e]
            if e == "act":
                nc.scalar.copy(dst, src)
            else:
                nc.vector.tensor_copy(dst, src)

        from concourse.ap import AP as _AP

        def pair_ap(base, delta):
            return _AP(tensor=base.tensor, offset=base.offset,
                       ap=[list(base.ap[0]), [delta, 2]] + [list(d) for d in base.ap[1:]])

        def tap_off(t):
            ky, kx = TAPS[t]
            return ky * PADW + kx

        # ---------------- phase A: blocks ----------------
        pending_gram = []
        for b in range(NB):
            par = b % 2
            r0 = b * R
            lo = max(r0 - 1, 0)
            hi = min(r0 + R, H - 1)
            nr = hi - lo + 1
            row_off = lo - (r0 - 1)  # 1 for b==0 else 0
            npx = nr * W

            if b == NB - 1:
                # zero the bottom halo row (stale from block b-2)
                for s in range(3):
                    nc.gpsimd.memset(padq[s][par][:, R + 1 : R + 2, :], 0.0)
                for s in range(2):
                    nc.gpsimd.memset(padv[s][par][:, R + 1 : R + 2, :], 0.0)

            x16 = xpool.tile([128, 2, npx], F16, tag="x16")
            x16_a = x16[:, 0, :]
            x16_b = x16[0:64, 1, :]
            x8t = xpool.tile([96, 2, npx], FP8, tag="x8t")
            # x8 first: the qk convs (first PE work of the block) need it
            nc.sync.dma_start(
                x8t[:],
                d_x8[:].rearrange("p (two n) -> p two n", two=2)[:, :, lo * W : (hi + 1) * W])
            nc.sync.dma_start(x16_a, d_x[0:128, lo * W : (hi + 1) * W])
            nc.sync.dma_start(x16_b, d_x[128:192, lo * W : (hi + 1) * W])
            if b == 0:
                deferred_const_dmas()

            # conv row chunking: 8-row (2-bank) PSUM tiles, matmuls 4 rows each
            chunks8 = [(i, min(8, nr - i)) for i in range(0, nr, 8)]

            def conv_qk(s):
                for (cr0, crn) in chunks8:
                    ps = ps_conv.tile([128, 8, W], F32, tag="conv")
                    for sub in range(0, crn, 4):
                        sn = min(4, crn - sub)
                        nc.tensor.matmul(
                            ps[:, sub : sub + sn, :], wqk8[:, :, 128 * s : 128 * (s + 1)],
                            x8t[:, :, (cr0 + sub) * W : (cr0 + sub + sn) * W],
                            start=True, stop=True, perf_mode=mybir.MatmulPerfMode.DoubleRow)
                    dstv = padq[s][par][:, row_off + cr0 : row_off + cr0 + crn, 2 : 2 + W]
                    emit_copy(dstv, ps[:, :crn, :], crn * W, force="act")

            def conv_v(s):
                mlo = 128 * s
                for (cr0, crn) in chunks8:
                    ps = ps_conv.tile([128, 8, W], F32, tag="conv")
                    for sub in range(0, crn, 4):
                        sn = min(4, crn - sub)
                        pss = ps[:, sub : sub + sn, :]
                        rhs_a = x16_a[:, (cr0 + sub) * W : (cr0 + sub + sn) * W]
                        rhs_b = x16_b[:, (cr0 + sub) * W : (cr0 + sub + sn) * W]
                        nc.tensor.matmul(pss, wvT_a[:, mlo : mlo + 128], rhs_a,
                                         start=True, stop=False)
                        nc.tensor.matmul(pss, wvT_b[:, mlo : mlo + 128], rhs_b,
                                         start=False, stop=True)
                    dstv = padv[s][par][:, row_off + cr0 : row_off + cr0 + crn, 2 : 2 + W]
                    # last block: DVE is nearly free, and draining ACT sooner
                    # lets the final transposes + gram + phase B start earlier
                    emit_copy(dstv, ps[:, :crn, :], crn * W,
                              force="dve" if b == NB - 1 else "act")

            qk_blk = [qkbpool.tile([128, PXB], F16, tag=f"qkblk{s}", name=f"qkblk{s}_{b}")
                      for s in range(3)]

            def dw_qk(s):
                dg = diag8[s]
                for ci in range(PXB // 512):
                    ps = ps_dw.tile([128, 4, W], F32, tag="pedw")
                    for pi, (t0, t1) in enumerate(DW_PAIRS):
                        ky, kx = TAPS[t0]
                        w0 = padq[s][par][:, ky + 4 * ci : ky + 4 * ci + 4, 1 + kx : 1 + kx + W]
                        nc.tensor.matmul(ps, dg[:, 2 * pi : 2 * pi + 2, :],
                                         pair_ap(w0, tap_off(t1) - tap_off(t0)),
                                         start=(pi == 0), stop=False,
                                         perf_mode=mybir.MatmulPerfMode.DoubleRow)
                    ky, kx = TAPS[4]
                    wc = padq[s][par][:, ky + 4 * ci : ky + 4 * ci + 4, 1 + kx : 1 + kx + W]
                    nc.tensor.matmul(ps, dg[:, 8:10, :], pair_ap(wc, 2),
                                     start=False, stop=True,
                                     perf_mode=mybir.MatmulPerfMode.DoubleRow)
                    dstv = qk_blk[s][:, 512 * ci : 512 * (ci + 1)]
                    emit_copy(dstv.rearrange("p (r c) -> p r c", r=4), ps, 512, force="act")

            # PE program order interleaves work that does not depend on fresh
            # copies (prev-block gram, depthwise of already-copied slabs) with
            # the conv matmuls, so the PE doesn't stall while ACT drains the
            # conv->pad copies.
            NG = PXB // 128
            qkT = blkpool.tile([128, NG * O], F16, tag="qkT")
            qkTv = qkT[:].rearrange("p (g o) -> p g o", g=NG)

            def transpose_slab(s):
                # issue from the ACT queue: the qk-dw copies it waits on run on
                # ACT, so the wait resolves in-order instead of head-of-line
                # blocking the SP DMA queue. Emitted per-slab right after that
                # slab's dw copies so the xbar transfers spread across the
                # block instead of serializing on DMA_ENGINES at its end.
                nc.scalar.dma_start_transpose(
                    qkTv[:, :, 128 * s : 128 * (s + 1)], qk_blk[s][:, 0:PXB]
                )

            conv_qk(0)
            conv_qk(1)
            conv_qk(2)
            conv_v(0)
            conv_v(1)
            # prev-block gram fills the PE while ACT drains the conv copies
            # (it depends only on last block's transpose, not fresh copies)
            if len(pending_gram) > 1:
                pending_gram.pop(0)()
            dw_qk(0)
            transpose_slab(0)
            dw_qk(1)
            transpose_slab(1)
            dw_qk(2)
            transpose_slab(2)

            def make_gram(qkT, b):
                def emit():
                    for g in range(NG):
                        for h in range(HEADS):
                            sl = qkT[:, g * O + 96 * h : g * O + 96 * (h + 1)]
                            nc.tensor.matmul(
                                gram[:, 96 * h : 96 * (h + 1)], sl, sl,
                                start=(b == 0 and g == 0),
                                stop=(b == NB - 1 and g == NG - 1),
                                skip_group_check=True,
                            )
                return emit

            pending_gram.append(make_gram(qkT, b))

            # v depthwise: PE share (fp16 diag matmuls -> PSUM -> copy seeds
            # v_buf), then remaining taps accumulate on DVE in-place.
            # Last block: everything on PE — the DVE tap chain would otherwise
            # drain alone after phase A while the PE sits idle.
            # Last block: slab0 fully on PE; slab1 keeps 3 DVE taps, which
            # drain hidden under the gram flush + phase B window.
            pe_taps = [list(range(9)), list(range(6))] if b == NB - 1 else V_PE_TAPS
            vsl = [v_buf[s][b][:] for s in range(2)]
            for s in range(2):
                tl = pe_taps[s]
                for ci in range(PXB // 512):
                    ps = ps_dw.tile([128, 4, W], F32, tag="pedw")
                    for j, ti in enumerate(tl):
                        ky, kx = TAPS[ti]
                        rhs = padv[s][par][:, ky + 4 * ci : ky + 4 * ci + 4, 1 + kx : 1 + kx + W]
                        nc.tensor.matmul(
                            ps, diagv[s][:, ti, :], rhs,
                            start=(j == 0), stop=(j == len(tl) - 1),
                        )
                    dstv = vsl[s][:, 512 * ci : 512 * (ci + 1)]
                    emit_copy(dstv.rearrange("p (r c) -> p r c", r=4), ps, 512, force="dve")
            for s in range(2):
                dvet = [t for t in range(9) if t not in pe_taps[s]]
                for j, t in enumerate(dvet):
                    ky, kx = TAPS[t]
                    w3 = padv[s][par][:, ky : ky + R, 1 + kx : 1 + kx + W]
                    w_ap = dwv_sc[s][:, t : t + 1]
                    tmp = dwsc[:, 0:PXB]
                    nc.vector.tensor_scalar(
                        tmp.rearrange("p (r c) -> p r c", r=R), w3, w_ap, None, OP.mult)
                    nc.vector.tensor_add(vsl[s], tmp, vsl[s])
                    eng_load["dve"] += 0.26 * PXB + 150 + 0.52 * PXB + 150

        while pending_gram:
            pending_gram.pop(0)()

        # ---------------- phase B ----------------
        ps_dw.release()
        ps_conv.release()
        ps_misc = tc.alloc_tile_pool(name="ps_misc", bufs=2, space="PSUM")

        # read the gram straight out of PSUM (it stays resident); skipping the
        # SBUF staging copy shortens the serial phase-B chain
        msk = pb_pool.tile([96, 4 * 96], F32, tag="msk")
        nc.vector.tensor_mul(msk[:], gram[:], gmask[:])
        ss = pb_pool.tile([96, 4], F32, tag="ss")
        nc.vector.tensor_reduce(
            ss[:], msk[:].rearrange("p (h n) -> p h n", h=4), mybir.AxisListType.X, OP.add
        )
        # rs = 1/sqrt(ss) = exp(-0.5*ln(ss)) — stays within the preloaded
        # natural_log_exp table set (no mid-phase table swap)
        rs = pb_pool.tile([96, 4], F32, tag="rs")
        nc.scalar.activation(rs[:], ss[:], AF.Ln)
        nc.scalar.activation(rs[:], rs[:], AF.Exp, scale=-0.5)
        nc.vector.tensor_mul(rs[:], rs[:], tmmask[:])  # fold temperature into q rows

        # row form of rs: [4, 96]
        ps_t = ps_misc.tile([128, 128], F32, tag="ps_misc")
        nc.tensor.transpose(ps_t[:4, :96], rs[:], idf32[:96, :96])
        rs_row = pb_pool.tile([4, 96], F32, tag="rs_row")
        nc.vector.tensor_copy(rs_row[:], ps_t[:4, :96])

        # column-scale tensor via selector matmuls: cs[h][c,d] = rs_k[h][d]
        ps_cs = ps_misc.tile([48, 4 * 48], F32, tag="ps_misc")
        for h in range(HEADS):
            nc.tensor.matmul(
                ps_cs[:, 48 * h : 48 * (h + 1)], hsel[:, 48 * h : 48 * (h + 1)],
                rs_row[:, 48:96], start=True, stop=True,
            )

        # S = G_qk * rs_q*temp (rows) * rs_k (cols)
        S = pb_pool.tile([48, 4 * 48], F32, tag="S")
        for h in range(HEADS):
            nc.vector.tensor_scalar(
                S[:, 48 * h : 48 * (h + 1)],
                gram[0:48, 96 * h + 48 : 96 * h + 96],
                rs[0:48, h : h + 1],
                None, OP.mult,
            )
        nc.vector.tensor_mul(S[:], S[:], ps_cs[:])
        P = pb_pool.tile([48, 4 * 48], F32, tag="P")
        nc.scalar.activation(P[:], S[:], AF.Exp)
        den = pb_pool.tile([48, 4], F32, tag="den")
        nc.vector.tensor_reduce(
            den[:], P[:].rearrange("p (h n) -> p h n", h=4), mybir.AxisListType.X, OP.add
        )
        nc.vector.reciprocal(den[:], den[:])
        A = pb_pool.tile([48, 4 * 48], F16, tag="A")
        for h in range(HEADS):
            nc.vector.tensor_scalar(
                A[:, 48 * h : 48 * (h + 1)], P[:, 48 * h : 48 * (h + 1)],
                den[:, h : h + 1], None, OP.mult,
            )
        # fused attn+proj weights: MhT[d, o] = sum_c A_h[c, d] * WpT[48h+c, o],
        # stored with head h at rows 64*(h%2), col block h//2 (matches v_buf)
        MhT = pb_pool.tile([128, 2, C], F16, tag="MhT")
        nc.vector.memset(MhT[:], 0.0)
        for h in range(HEADS):
            ps_m = ps_misc.tile([128, C], F32, tag="ps_m", name=f"ps_m{h}")
            rlo = 64 * (h % 2)
            nc.tensor.matmul(
                ps_m[rlo : rlo + 48, :], A[:, 48 * h : 48 * (h + 1)],
                wphead[:, h, :], start=True, stop=True,
            )
            nc.scalar.copy(MhT[rlo : rlo + 48, h // 2, :], ps_m[rlo : rlo + 48, :])

        # fused attn@proj @ v -> out, in 512-px chunks
        ps_misc.release()
        ps_gram.release()
        ps_o = tc.alloc_tile_pool(name="ps_o", bufs=4, space="PSUM")
        NCH = HW // 512
        GRP = 2  # chunks per y DMA (fewer, larger DMAs: HWDGE dispatch is 625ns each)
        ys_a = ys_b = None
        for ci in range(NCH):
            px = ci * 512
            g = ci % GRP
            if g == 0:
                ys_a = ys_pool.tile([128, GRP, 512], F16, tag="ys_a")
                ys_b = ys_pool.tile([64, GRP, 512], F16, tag="ys_b")
            py_a = ps_o.tile([128, 512], F32, tag="py_a")
            py_b = ps_o.tile([64, 512], F32, tag="py_b")
            for s in range(2):
                rhs = v_buf[s][ci // 4][:, (ci % 4) * 512 : (ci % 4 + 1) * 512]
                nc.tensor.matmul(py_a[:], MhT[:, s, 0:128], rhs,
                                 start=(s == 0), stop=(s == 1))
                nc.tensor.matmul(py_b[:], MhT[:, s, 128:192], rhs,
                                 start=(s == 0), stop=(s == 1))
            emit_copy(ys_a[:, g, :], py_a[:], 512, force="act" if ci % 2 == 0 else "dve")
            emit_copy(ys_b[:, g, :], py_b[:], 512, force="dve" if ci % 2 == 0 else "act")
            if g == GRP - 1:
                p0 = (ci - g) * 512
                nc.sync.dma_start(d_y[0:128, p0 : p0 + GRP * 512], ys_a[:])
                nc.sync.dma_start(d_y[128:192, p0 : p0 + GRP * 512], ys_b[:])

        ps_o.release()
        ys_pool.release()
        pb_pool.release()
        scpool.release()
        qkbpool.release()
        blkpool.release()
        xpool.release()
        persist.release()
        consts.release()

    nc.compile()
    return nc


# ---------------- host side ----------------
_CACHE = {}


def _prep_static(W_qk, W_qk_dw, W_v, W_v_dw, W_proj, temperature):
    # head-interleaved channel permutation for qk: [q_h|k_h] blocks of 96
    perm = np.zeros(O, np.int64)
    for h in range(HEADS):
        perm[96 * h : 96 * h + 48] = np.arange(48 * h, 48 * h + 48)
        perm[96 * h + 48 : 96 * h + 96] = 192 + np.arange(48 * h, 48 * h + 48)

    wqkT = np.ascontiguousarray(W_qk[:, :, 0, 0].T[:, perm]).astype(np.float32)
    dwqk = np.ascontiguousarray(W_qk_dw[:, 0].reshape(O, 9)[perm]).astype(np.float32)

    # v channels padded to 64-aligned head slots: new chan (s,r): head 2s+r//64,
    # within-head idx r%64 (<48 live, else dead/zero). 256 slots = 2 slabs x 128.
    live = np.zeros(VSLOTS, np.bool_)
    src_ch = np.zeros(VSLOTS, np.int64)
    for s in range(2):
        for j in range(2):
            h = 2 * s + j
            r = 128 * s + 64 * j
            live[r : r + 48] = True
            src_ch[r : r + 48] = 48 * h + np.arange(48)

    wvT_orig = W_v[:, :, 0, 0].T.astype(np.float32)   # [192 in, 192 out]
    wvT = np.zeros((C, VSLOTS), np.float32)
    wvT[:, live] = wvT_orig[:, src_ch[live]]

    dwv_orig = W_v_dw[:, 0].reshape(C, 9).astype(np.float32)
    dwv = np.zeros((VSLOTS, 9), np.float32)
    dwv[live] = dwv_orig[src_ch[live]]
    dwv_sc = np.stack([dwv[0:128], dwv[128:256]], 0).transpose(1, 0, 2).reshape(128, 18)

    diagv0 = np.zeros((128, 9, 128), np.float32)
    diagv1 = np.zeros((128, 9, 128), np.float32)
    for t in range(9):
        diagv0[np.arange(128), t, np.arange(128)] = dwv[0:128, t]
        diagv1[np.arange(128), t, np.arange(128)] = dwv[128:256, t]

    # qk conv weights DoubleRow-packed
    wqk8 = np.zeros((96, 2, O), np.float32)
    wqk8[:, 0, :] = wqkT[0:96, :]
    wqk8[:, 1, :] = wqkT[96:192, :]

    # fp8 diag depthwise matrices for the 3 qk slabs, DR slot layout
    _slot = {}
    for _pi, (_t0, _t1) in enumerate(DW_PAIRS):
        _slot[2 * _pi] = _t0
        _slot[2 * _pi + 1] = _t1
    _slot[8] = 4                                        # center tap; slot 9 stays zero
    diag8 = np.zeros((128, 3, 10, 128), np.float32)
    for _s in range(3):
        for _sl, _t in _slot.items():
            diag8[np.arange(128), _s, _sl, np.arange(128)] = dwqk[128 * _s : 128 * (_s + 1), _t]

    # proj weights per head: wphead[c, h, o] = WpT[48h+c, o]
    wprojT_orig = W_proj[:, :, 0, 0].T.astype(np.float32)  # [192 in, 192 out]
    wphead = np.zeros((48, 4, C), np.float32)
    for h in range(HEADS):
        wphead[:, h, :] = wprojT_orig[48 * h : 48 * (h + 1), :]

    gmask = np.zeros((96, 4 * 96), np.float32)
    for h in range(HEADS):
        gmask[np.arange(96), 96 * h + np.arange(96)] = 1.0

    temp = np.asarray(temperature).reshape(HEADS)
    tmmask = np.ones((96, 4), np.float32)
    tmmask[0:48, :] = temp[None, :]

    hsel = np.zeros((4, 4 * 48), np.float32)
    for h in range(HEADS):
        hsel[h, 48 * h : 48 * (h + 1)] = 1.0

    return {
        "wqk8": wqk8.reshape(96, 2 * O).astype(fp8),
        "wvT": wvT.astype(fp16),
        "wphead": wphead.reshape(48, 4 * C).astype(fp16),
        "diag8": diag8.reshape(128, 3 * 10 * 128).astype(fp8),
        "diagv0": diagv0.reshape(128, 9 * 128).astype(fp16),
        "diagv1": diagv1.reshape(128, 9 * 128).astype(fp16),
        "dwv_sc": dwv_sc.astype(np.float32),
        "gmask": gmask,
        "tmmask": tmmask,
        "idf32": np.eye(128, dtype=np.float32),
        "hsel": hsel,
    }


def kernel(x, W_qk, W_qk_dw, W_v, W_v_dw, W_proj, temperature):
    x = np.asarray(x, np.float32)
    b = x.shape[0]
    assert b == 8 and x.shape[1] == C

    if "nc" not in _CACHE:
        _CACHE["nc"] = build_nc()
    nc = _CACHE["nc"]

    static = _prep_static(
        np.asarray(W_qk), np.asarray(W_qk_dw), np.asarray(W_v),
        np.asarray(W_v_dw), np.asarray(W_proj), np.asarray(temperature),
    )
    in_maps = []
    for i in range(b):
        m = dict(static)
        xi = np.ascontiguousarray(x[i].reshape(C, HW))
        m["x"] = xi.astype(fp16)
        x8 = np.zeros((96, 2, HW), np.float32)
        x8[:, 0, :] = xi[0:96]
        x8[:, 1, :] = xi[96:192]
        m["x8"] = x8.reshape(96, 2 * HW).astype(fp8)
        in_maps.append(m)

    res = run_bass_kernel_spmd(nc, in_maps, core_ids=list(range(8)))
    y = np.stack([res.results[i]["y"].reshape(C, H, W) for i in range(8)])
    return y.astype(np.float32)


if __name__ == "__main__":
    t0 = time.time()
    nc = build_nc()
    print(f"build+compile: {time.time()-t0:.1f}s")


# revision 79
# speedup vs baseline: 1.1737x; 1.1737x over previous
# Trainium2 Bass kernel for nn_Attention_57509612094021 (XCA-style channel attention).
#
# Sharding: pure data-parallel over batch (8 images -> 8 NeuronCores), no collectives.
# Per-core pipeline (one [192,128,128] image):
#   - qk path all-fp8: 1x1 conv as fp8 DoubleRow matmuls (3 slabs of 128),
#     PSUM -> fp8 zero-padded SBUF slabs, depthwise 3x3 on PE as fp8
#     DoubleRow diag-matmuls (4 tap-pairs + center), PSUM -> fp16 qk_blk,
#   - v path fp16 (v feeds the output linearly, fp8 fails the error budget):
#     1x1 conv fp16 (2-pass contraction 128+64) -> fp16 padded slabs;
#     depthwise 3x3 WITHOUT PSUM: accumulated in-place into v_buf by DVE
#     (tensor_scalar mult + tensor_tensor add pairs) and the otherwise-idle
#     Pool/GpSimd engine (fused scalar_tensor_tensor mult-add), chains
#     anti-phased across the two slabs so both engines stay busy,
#   - PSUM->SBUF copies greedily balanced across ACT/DVE by a load model,
#   - per-block batched DMA-xbar transposes of q|k -> stacked per-head [96,96]
#     Gram accumulated in PSUM across the whole image (diag = L2 norms),
#   - rsqrt + Newton step, temperature/row/col scaling, softmax (exp on ACT),
#     proj FUSED into attention: M_h = A_h @ Wp_h, y = sum_h M_h^T.T @ v_h.
# Host side pre-permutes weights (head-interleaved qk, 64-aligned v slots) and
# builds the fp8 diagonal depthwise matrices, masks, and selector matrices.
import os
import sys
import time

sys.path.insert(0, "/opt/trn_rl_repo")
os.environ.setdefault("JAX_PLATFORMS", "axon")

import numpy as np
import ml_dtypes

import concourse.bass as bass
import concourse.tile as tile
from concourse import bacc, mybir
from concourse.bass_utils import run_bass_kernel_spmd

# Prefer the act-table set that covers BOTH Ln and Exp: the load-insertion
# pass picks the first covering set per func, and with the default tables it
# alternates exp_and_others / natural_log_exp_and_others (4 swaps). Blank out
# exp_and_others (set ids are positional, so order must be preserved) so Exp
# resolves to natural_log_exp_and_others and the kernel lives in one set.
_orig_get_activation_tables = bacc.get_activation_tables

def _get_activation_tables_ln_exp_combined(arch):
    t = dict(_orig_get_activation_tables(arch))
    t["exp_and_others"] = set()
    return t

bacc.get_activation_tables = _get_activation_tables_ln_exp_combined

F32 = mybir.dt.float32
F32R = mybir.dt.float32r
BF16 = mybir.dt.bfloat16
F16 = mybir.dt.float16
FP8 = mybir.dt.float8e4
AF = mybir.ActivationFunctionType
OP = mybir.AluOpType
bf16 = ml_dtypes.bfloat16
fp16 = np.float16
fp8 = ml_dtypes.float8_e4m3

C = 192
O = 384          # 2C (qk out channels)
VSLOTS = 256     # v out slots (64-aligned heads, 192 live)
H = W = 128
HW = H * W
HEADS = 4
CPH = 48
R = 16           # rows per block
NB = H // R      # 8 blocks
PXB = R * W      # 2048 pixels per block
TAPS = [(ky, kx) for ky in range(3) for kx in range(3)]
DW_PAIRS = [(0, 2), (3, 5), (6, 8), (1, 7)]  # DoubleRow tap pairs; center 4 alone

# v depthwise tap split per slab: PE (fp16 diag matmuls, seeds v_buf via the
# PSUM copy) + DVE (tensor_scalar mult + tensor_tensor add pairs).
V_PE_TAPS = [[0, 1, 2, 3], [0, 1, 2, 3]]


def build_nc():
    nc = bacc.Bacc("TRN2", target_bir_lowering=False, debug=False, num_devices=8)

    d_x = nc.dram_tensor("x", [C, HW], F16, kind="ExternalInput").ap()
    d_x8 = nc.dram_tensor("x8", [96, 2 * HW], FP8, kind="ExternalInput").ap()
    d_wqk8 = nc.dram_tensor("wqk8", [96, 2 * O], FP8, kind="ExternalInput").ap()
    d_wvT = nc.dram_tensor("wvT", [C, VSLOTS], F16, kind="ExternalInput").ap()
    d_wphead = nc.dram_tensor("wphead", [48, 4 * C], F16, kind="ExternalInput").ap()
    d_diag8 = nc.dram_tensor("diag8", [128, 3 * 10 * 128], FP8, kind="ExternalInput").ap()
    d_diagv0 = nc.dram_tensor("diagv0", [128, 9 * 128], F16, kind="ExternalInput").ap()
    d_diagv1 = nc.dram_tensor("diagv1", [128, 9 * 128], F16, kind="ExternalInput").ap()
    d_dwv_sc = nc.dram_tensor("dwv_sc", [128, 18], F32, kind="ExternalInput").ap()
    d_gmask = nc.dram_tensor("gmask", [96, 4 * 96], F32, kind="ExternalInput").ap()
    d_tmmask = nc.dram_tensor("tmmask", [96, 4], F32, kind="ExternalInput").ap()
    d_idf32 = nc.dram_tensor("idf32", [128, 128], F32, kind="ExternalInput").ap()
    d_hsel = nc.dram_tensor("hsel", [4, 4 * 48], F32, kind="ExternalInput").ap()
    d_y = nc.dram_tensor("y", [C, HW], F16, kind="ExternalOutput").ap()

    with tile.TileContext(nc) as tc:
        # ---------------- pools ----------------
        consts = tc.alloc_tile_pool(name="consts", bufs=1)
        persist = tc.alloc_tile_pool(name="persist", bufs=1)
        xpool = tc.alloc_tile_pool(name="xpool", bufs=2)
        blkpool = tc.alloc_tile_pool(name="blkpool", bufs=2)
        qkbpool = tc.alloc_tile_pool(name="qkbpool", bufs=2)
        scpool = tc.alloc_tile_pool(name="scpool", bufs=1)
        pb_pool = tc.alloc_tile_pool(name="pbpool", bufs=1)
        ys_pool = tc.alloc_tile_pool(name="yspool", bufs=2)
        ps_gram = tc.alloc_tile_pool(name="ps_gram", bufs=1, space="PSUM")
        ps_conv = tc.alloc_tile_pool(name="ps_conv", bufs=2, space="PSUM")  # 2-bank tiles
        ps_dw = tc.alloc_tile_pool(name="ps_dw", bufs=3, space="PSUM")

        # ---------------- constants / weights ----------------
        wqk8 = consts.tile([96, 2, O], FP8, tag="wqk8")
        wvT_a = consts.tile([128, VSLOTS], F16, tag="wvT_a")
        wvT_b = consts.tile([64, VSLOTS], F16, tag="wvT_b")
        wphead = consts.tile([48, 4, C], F16, tag="wphead")
        diag8 = [consts.tile([128, 10, 128], FP8, tag=f"diag8_{s}", name=f"diag8_{s}")
                 for s in range(3)]
        diagv = [consts.tile([128, 9, 128], F16, tag=f"diagv{s}", name=f"diagv{s}")
                 for s in range(2)]
        dwv_sc = [consts.tile([128, 9], F32, tag=f"dwvsc{s}", name=f"dwvsc{s}") for s in range(2)]
        gmask = consts.tile([96, 4 * 96], F32, tag="gmask")
        tmmask = consts.tile([96, 4], F32, tag="tmmask")
        idf32 = consts.tile([128, 128], F32, tag="idf32")
        hsel = consts.tile([4, 4 * 48], F32, tag="hsel")

        # conv-critical consts first; the rest are deferred behind block 0's
        # input DMAs so the PE can start ~8us earlier.
        nc.sync.dma_start(wqk8[:], d_wqk8[:].rearrange("p (two o) -> p two o", two=2))

        def deferred_const_dmas():
            nc.sync.dma_start(wvT_a[:], d_wvT[0:128, :])
            nc.sync.dma_start(wvT_b[:], d_wvT[128:192, :])
            for s in range(3):
                nc.sync.dma_start(
                    diag8[s][:],
                    d_diag8[:, 1280 * s : 1280 * (s + 1)].rearrange("p (t c) -> p t c", t=10))
            nc.sync.dma_start(diagv[0][:], d_diagv0[:].rearrange("p (t c) -> p t c", t=9))
            nc.sync.dma_start(diagv[1][:], d_diagv1[:].rearrange("p (t c) -> p t c", t=9))
            for s in range(2):
                nc.sync.dma_start(dwv_sc[s][:], d_dwv_sc[:, 9 * s : 9 * (s + 1)])
            nc.sync.dma_start(wphead[:], d_wphead[:].rearrange("p (h o) -> p h o", h=4))
            nc.sync.dma_start(gmask[:], d_gmask[:])
            nc.sync.dma_start(tmmask[:], d_tmmask[:])
            nc.sync.dma_start(idf32[:], d_idf32[:])
            nc.sync.dma_start(hsel[:], d_hsel[:])

        # Preload the ONE act table set phase B needs: natural_log_exp covers
        # both Ln and Exp (rsqrt is computed as exp(-0.5*ln)), so no
        # LoadActFuncSet ever lands on the phase-B critical path.
        actwarm = consts.tile([1, 8], F32, tag="actwarm")
        nc.vector.memset(actwarm[:], 1.0)
        nc.scalar.activation(actwarm[:], actwarm[:], AF.Ln)
        nc.scalar.activation(actwarm[:], actwarm[:], AF.Exp)

        # ---------------- persistent tensors ----------------
        # v_buf split into per-block tiles so the dependency tracker lets the
        # out phase start on early blocks while late-block taps still run
        v_buf = [
            [persist.tile([128, PXB], F16, tag=f"v_buf{s}_{bb}", name=f"v_buf{s}_{bb}")
             for bb in range(NB)]
            for s in range(2)
        ]
        gram = ps_gram.tile([96, 4 * 96], F32, tag="gram")

        # padded conv-output slabs, manual parity double-buffer
        PADW = 132
        NPR = R + 2
        padq = [[persist.tile([128, NPR, PADW], FP8, tag=f"pq{s}{p}", name=f"pq{s}{p}")
                 for p in range(2)] for s in range(3)]
        padv = [[persist.tile([128, NPR, PADW], F16, tag=f"pv{s}{p}", name=f"pv{s}{p}")
                 for p in range(2)] for s in range(2)]
        # zero the side columns once (cols 0,1,130,131 never written later)
        for grp in (padq, padv):
            for row in grp:
                for t in row:
                    nc.gpsimd.memset(t[:, :, 0:2], 0.0)
                    nc.gpsimd.memset(t[:, :, 130:132], 0.0)
        # zero halo rows used by first block (parity 0)
        for grp in (padq, padv):
            for row in grp:
                nc.gpsimd.memset(row[0][:, 0:1, :], 0.0)

        # DVE scratch for v-depthwise tap products (one buffer: DVE is a
        # serial FIFO engine, ping-pong would buy nothing)
        dwsc = scpool.tile([128, PXB], F16, tag="dwsc")
        # Pool-engine scratch (half-block sized) for its broadcast-mult taps
        plsc = scpool.tile([128, PXB // 2], F16, tag="plsc")

        # ---------- balanced copy emitter (ACT / DVE; Pool can't read PSUM) ----------
        eng_load = {"act": 0.0, "dve": 0.0}
        RATE = {"act": 0.833, "dve": 1.042}
        OVH = {"act": 150.0, "dve": 130.0}

        def emit_copy(dst, src, elems, force=None):
            if force is not None:
                e = force
            else:
                e = min(eng_load, key=lambda k: eng_load[k] + RATE[k] * elems + OVH[k])
            eng_load[e] += RATE[e] * elems + OVH[e]
            if e == "act":
                nc.scalar.copy(dst, src)
            else:
                nc.vector.tensor_copy(dst, src)

        from concourse.ap import AP as _AP

        def pair_ap(base, delta):
            return _AP(tensor=base.tensor, offset=base.offset,
                       ap=[list(base.ap[0]), [delta, 2]] + [list(d) for d in base.ap[1:]])

        def tap_off(t):
            ky, kx = TAPS[t]
            return ky * PADW + kx

        # ---------------- phase A: blocks ----------------
        pending_gram = []
        for b in range(NB):
            par = b % 2
            r0 = b * R
            lo = max(r0 - 1, 0)
            hi = min(r0 + R, H - 1)
            nr = hi - lo + 1
            row_off = lo - (r0 - 1)  # 1 for b==0 else 0
            npx = nr * W

            if b == NB - 1:
                # zero the bottom halo row (stale from block b-2)
                for s in range(3):
                    nc.gpsimd.memset(padq[s][par][:, R + 1 : R + 2, :], 0.0)
                for s in range(2):
                    nc.gpsimd.memset(padv[s][par][:, R + 1 : R + 2, :], 0.0)

            x16 = xpool.tile([128, 2, npx], F16, tag="x16")
            x16_a = x16[:, 0, :]
            x16_b = x16[0:64, 1, :]
            x8t = xpool.tile([96, 2, npx], FP8, tag="x8t")
            # x8 first: the qk convs (first PE work of the block) need it
            nc.sync.dma_start(
                x8t[:],
                d_x8[:].rearrange("p (two n) -> p two n", two=2)[:, :, lo * W : (hi + 1) * W])
            nc.sync.dma_start(x16_a, d_x[0:128, lo * W : (hi + 1) * W])
            nc.sync.dma_start(x16_b, d_x[128:192, lo * W : (hi + 1) * W])
            if b == 0:
                deferred_const_dmas()

            # conv row chunking: 8-row (2-bank) PSUM tiles, matmuls 4 rows each
            chunks8 = [(i, min(8, nr - i)) for i in range(0, nr, 8)]

            def conv_qk(s):
                for (cr0, crn) in chunks8:
                    ps = ps_conv.tile([128, 8, W], F32, tag="conv")
                    for sub in range(0, crn, 4):
                        sn = min(4, crn - sub)
                        nc.tensor.matmul(
                            ps[:, sub : sub + sn, :], wqk8[:, :, 128 * s : 128 * (s + 1)],
                            x8t[:, :, (cr0 + sub) * W : (cr0 + sub + sn) * W],
                            start=True, stop=True, perf_mode=mybir.MatmulPerfMode.DoubleRow)
                    dstv = padq[s][par][:, row_off + cr0 : row_off + cr0 + crn, 2 : 2 + W]
                    emit_copy(dstv, ps[:, :crn, :], crn * W, force="act")

            def conv_v(s):
                mlo = 128 * s
                for (cr0, crn) in chunks8:
                    ps = ps_conv.tile([128, 8, W], F32, tag="conv")
                    for sub in range(0, crn, 4):
                        sn = min(4, crn - sub)
                        pss = ps[:, sub : sub + sn, :]
                        rhs_a = x16_a[:, (cr0 + sub) * W : (cr0 + sub + sn) * W]
                        rhs_b = x16_b[:, (cr0 + sub) * W : (cr0 + sub + sn) * W]
                        nc.tensor.matmul(pss, wvT_a[:, mlo : mlo + 128], rhs_a,
                                         start=True, stop=False)
                        nc.tensor.matmul(pss, wvT_b[:, mlo : mlo + 128], rhs_b,
                                         start=False, stop=True)
                    dstv = padv[s][par][:, row_off + cr0 : row_off + cr0 + crn, 2 : 2 + W]
                    # last block: DVE is nearly free, and draining ACT sooner
                    # lets the final transposes + gram + phase B start earlier
                    emit_copy(dstv, ps[:, :crn, :], crn * W,
                              force="dve" if b == NB - 1 else "act")

            qk_blk = [qkbpool.tile([128, PXB], F16, tag=f"qkblk{s}", name=f"qkblk{s}_{b}")
                      for s in range(3)]

            def dw_qk(s):
                dg = diag8[s]
                for ci in range(PXB // 512):
                    ps = ps_dw.tile([128, 4, W], F32, tag="pedw")
                    for pi, (t0, t1) in enumerate(DW_PAIRS):
                        ky, kx = TAPS[t0]
                        w0 = padq[s][par][:, ky + 4 * ci : ky + 4 * ci + 4, 1 + kx : 1 + kx + W]
                        nc.tensor.matmul(ps, dg[:, 2 * pi : 2 * pi + 2, :],
                                         pair_ap(w0, tap_off(t1) - tap_off(t0)),
                                         start=(pi == 0), stop=False,
                                         perf_mode=mybir.MatmulPerfMode.DoubleRow)
                    ky, kx = TAPS[4]
                    wc = padq[s][par][:, ky + 4 * ci : ky + 4 * ci + 4, 1 + kx : 1 + kx + W]
                    nc.tensor.matmul(ps, dg[:, 8:10, :], pair_ap(wc, 2),
                                     start=False, stop=True,
                                     perf_mode=mybir.MatmulPerfMode.DoubleRow)
                    dstv = qk_blk[s][:, 512 * ci : 512 * (ci + 1)]
                    emit_copy(dstv.rearrange("p (r c) -> p r c", r=4), ps, 512, force="act")

            # PE program order interleaves work that does not depend on fresh
            # copies (prev-block gram, depthwise of already-copied slabs) with
            # the conv matmuls, so the PE doesn't stall while ACT drains the
            # conv->pad copies.
            conv_qk(0)
            conv_qk(1)
            conv_qk(2)
            conv_v(0)
            conv_v(1)
            # prev-block gram fills the PE while ACT drains the conv copies
            # (it depends only on last block's transpose, not fresh copies)
            if len(pending_gram) > 1:
                pending_gram.pop(0)()
            dw_qk(0)
            dw_qk(1)
            dw_qk(2)

            # transpose q|k block -> [px, 384] groups via DMA xbar; the gram
            # matmuls are deferred one block so the PE never waits on the DMA.
            NG = PXB // 128
            qkT = blkpool.tile([128, NG * O], F16, tag="qkT")
            qkTv = qkT[:].rearrange("p (g o) -> p g o", g=NG)
            for s in range(3):
                # issue from the ACT queue: the qk-dw copies it waits on run on
                # ACT, so the wait resolves in-order instead of head-of-line
                # blocking the SP DMA queue.
                nc.scalar.dma_start_transpose(
                    qkTv[:, :, 128 * s : 128 * (s + 1)], qk_blk[s][:, 0:PXB]
                )

            def make_gram(qkT, b):
                def emit():
                    for g in range(NG):
                        for h in range(HEADS):
                            sl = qkT[:, g * O + 96 * h : g * O + 96 * (h + 1)]
                            nc.tensor.matmul(
                                gram[:, 96 * h : 96 * (h + 1)], sl, sl,
                                start=(b == 0 and g == 0),
                                stop=(b == NB - 1 and g == NG - 1),
                                skip_group_check=True,
                            )
                return emit

            pending_gram.append(make_gram(qkT, b))

            # v depthwise: PE share (fp16 diag matmuls -> PSUM -> copy seeds
            # v_buf), then remaining taps accumulate on DVE in-place.
            # Last block: everything on PE — the DVE tap chain would otherwise
            # drain alone after phase A while the PE sits idle.
            # Last block: slab0 fully on PE; slab1 keeps 3 DVE taps, which
            # drain hidden under the gram flush + phase B window.
            pe_taps = [list(range(9)), list(range(6))] if b == NB - 1 else V_PE_TAPS
            vsl = [v_buf[s][b][:] for s in range(2)]
            for s in range(2):
                tl = pe_taps[s]
                for ci in range(PXB // 512):
                    ps = ps_dw.tile([128, 4, W], F32, tag="pedw")
                    for j, ti in enumerate(tl):
                        ky, kx = TAPS[ti]
                        rhs = padv[s][par][:, ky + 4 * ci : ky + 4 * ci + 4, 1 + kx : 1 + kx + W]
                        nc.tensor.matmul(
                            ps, diagv[s][:, ti, :], rhs,
                            start=(j == 0), stop=(j == len(tl) - 1),
                        )
                    dstv = vsl[s][:, 512 * ci : 512 * (ci + 1)]
                    emit_copy(dstv.rearrange("p (r c) -> p r c", r=4), ps, 512, force="dve")
            for s in range(2):
                dvet = [t for t in range(9) if t not in pe_taps[s]]
                for j, t in enumerate(dvet):
                    ky, kx = TAPS[t]
                    w3 = padv[s][par][:, ky : ky + R, 1 + kx : 1 + kx + W]
                    w_ap = dwv_sc[s][:, t : t + 1]
                    tmp = dwsc[:, 0:PXB]
                    nc.vector.tensor_scalar(
                        tmp.rearrange("p (r c) -> p r c", r=R), w3, w_ap, None, OP.mult)
                    nc.vector.tensor_add(vsl[s], tmp, vsl[s])
                    eng_load["dve"] += 0.26 * PXB + 150 + 0.52 * PXB + 150

        while pending_gram:
            pending_gram.pop(0)()

        # ---------------- phase B ----------------
        ps_dw.release()
        ps_conv.release()
        ps_misc = tc.alloc_tile_pool(name="ps_misc", bufs=2, space="PSUM")

        # read the gram straight out of PSUM (it stays resident); skipping the
        # SBUF staging copy shortens the serial phase-B chain
        msk = pb_pool.tile([96, 4 * 96], F32, tag="msk")
        nc.vector.tensor_mul(msk[:], gram[:], gmask[:])
        ss = pb_pool.tile([96, 4], F32, tag="ss")
        nc.vector.tensor_reduce(
            ss[:], msk[:].rearrange("p (h n) -> p h n", h=4), mybir.AxisListType.X, OP.add
        )
        # rs = 1/sqrt(ss) = exp(-0.5*ln(ss)) — stays within the preloaded
        # natural_log_exp table set (no mid-phase table swap)
        rs = pb_pool.tile([96, 4], F32, tag="rs")
        nc.scalar.activation(rs[:], ss[:], AF.Ln)
        nc.scalar.activation(rs[:], rs[:], AF.Exp, scale=-0.5)
        nc.vector.tensor_mul(rs[:], rs[:], tmmask[:])  # fold temperature into q rows

        # row form of rs: [4, 96]
        ps_t = ps_misc.tile([128, 128], F32, tag="ps_misc")
        nc.tensor.transpose(ps_t[:4, :96], rs[:], idf32[:96, :96])
        rs_row = pb_pool.tile([4, 96], F32, tag="rs_row")
        nc.vector.tensor_copy(rs_row[:], ps_t[:4, :96])

        # column-scale tensor via selector matmuls: cs[h][c,d] = rs_k[h][d]
        ps_cs = ps_misc.tile([48, 4 * 48], F32, tag="ps_misc")
        for h in range(HEADS):
            nc.tensor.matmul(
                ps_cs[:, 48 * h : 48 * (h + 1)], hsel[:, 48 * h : 48 * (h + 1)],
                rs_row[:, 48:96], start=True, stop=True,
            )

        # S = G_qk * rs_q*temp (rows) * rs_k (cols)
        S = pb_pool.tile([48, 4 * 48], F32, tag="S")
        for h in range(HEADS):
            nc.vector.tensor_scalar(
                S[:, 48 * h : 48 * (h + 1)],
                gram[0:48, 96 * h + 48 : 96 * h + 96],
                rs[0:48, h : h + 1],
                None, OP.mult,
            )
        nc.vector.tensor_mul(S[:], S[:], ps_cs[:])
        P = pb_pool.tile([48, 4 * 48], F32, tag="P")
        nc.scalar.activation(P[:], S[:], AF.Exp)
        den = pb_pool.tile([48, 4], F32, tag="den")
        nc.vector.tensor_reduce(
            den[:], P[:].rearrange("p (h n) -> p h n", h=4), mybir.AxisListType.X, OP.add
        )
        nc.vector.reciprocal(den[:], den[:])
        A = pb_pool.tile([48, 4 * 48], F16, tag="A")
        for h in range(HEADS):
            nc.vector.tensor_scalar(
                A[:, 48 * h : 48 * (h + 1)], P[:, 48 * h : 48 * (h + 1)],
                den[:, h : h + 1], None, OP.mult,
            )
        # fused attn+proj weights: MhT[d, o] = sum_c A_h[c, d] * WpT[48h+c, o],
        # stored with head h at rows 64*(h%2), col block h//2 (matches v_buf)
        MhT = pb_pool.tile([128, 2, C], F16, tag="MhT")
        nc.vector.memset(MhT[:], 0.0)
        for h in range(HEADS):
            ps_m = ps_misc.tile([128, C], F32, tag="ps_m", name=f"ps_m{h}")
            rlo = 64 * (h % 2)
            nc.tensor.matmul(
                ps_m[rlo : rlo + 48, :], A[:, 48 * h : 48 * (h + 1)],
                wphead[:, h, :], start=True, stop=True,
            )
            nc.scalar.copy(MhT[rlo : rlo + 48, h // 2, :], ps_m[rlo : rlo + 48, :])

        # fused attn@proj @ v -> out, in 512-px chunks
        ps_misc.release()
        ps_gram.release()
        ps_o = tc.alloc_tile_pool(name="ps_o", bufs=4, space="PSUM")
        NCH = HW // 512
        GRP = 2  # chunks per y DMA (fewer, larger DMAs: HWDGE dispatch is 625ns each)
        ys_a = ys_b = None
        for ci in range(NCH):
            px = ci * 512
            g = ci % GRP
            if g == 0:
                ys_a = ys_pool.tile([128, GRP, 512], F16, tag="ys_a")
                ys_b = ys_pool.tile([64, GRP, 512], F16, tag="ys_b")
            py_a = ps_o.tile([128, 512], F32, tag="py_a")
            py_b = ps_o.tile([64, 512], F32, tag="py_b")
            for s in range(2):
                rhs = v_buf[s][ci // 4][:, (ci % 4) * 512 : (ci % 4 + 1) * 512]
                nc.tensor.matmul(py_a[:], MhT[:, s, 0:128], rhs,
                                 start=(s == 0), stop=(s == 1))
                nc.tensor.matmul(py_b[:], MhT[:, s, 128:192], rhs,
                                 start=(s == 0), stop=(s == 1))
            emit_copy(ys_a[:, g, :], py_a[:], 512, force="act" if ci % 2 == 0 else "dve")
            emit_copy(ys_b[:, g, :], py_b[:], 512, force="dve" if ci % 2 == 0 else "act")
            if g == GRP - 1:
                p0 = (ci - g) * 512
                nc.sync.dma_start(d_y[0:128, p0 : p0 + GRP * 512], ys_a[:])
                nc.sync.dma_start(d_y[128:192, p0 : p0 + GRP * 512], ys_b[:])

        ps_o.release()
        ys_pool.release()
        pb_pool.release()
        scpool.release()
        qkbpool.release()
        blkpool.release()
        xpool.release()
        persist.release()
        consts.release()

    nc.compile()
    return nc


# ---------------- host side ----------------
_CACHE = {}


def _prep_static(W_qk, W_qk_dw, W_v, W_v_dw, W_proj, temperature):
    # head-interleaved channel permutation for qk: [q_h|k_h] blocks of 96
    perm = np.zeros(O, np.int64)
    for h in range(HEADS):
        perm[96 * h : 96 * h + 48] = np.arange(48 * h, 48 * h + 48)
        perm[96 * h + 48 : 96 * h + 96] = 192 + np.arange(48 * h, 48 * h + 48)

    wqkT = np.ascontiguousarray(W_qk[:, :, 0, 0].T[:, perm]).astype(np.float32)
    dwqk = np.ascontiguousarray(W_qk_dw[:, 0].reshape(O, 9)[perm]).astype(np.float32)

    # v channels padded to 64-aligned head slots: new chan (s,r): head 2s+r//64,
    # within-head idx r%64 (<48 live, else dead/zero). 256 slots = 2 slabs x 128.
    live = np.zeros(VSLOTS, np.bool_)
    src_ch = np.zeros(VSLOTS, np.int64)
    for s in range(2):
        for j in range(2):
            h = 2 * s + j
            r = 128 * s + 64 * j
            live[r : r + 48] = True
            src_ch[r : r + 48] = 48 * h + np.arange(48)

    wvT_orig = W_v[:, :, 0, 0].T.astype(np.float32)   # [192 in, 192 out]
    wvT = np.zeros((C, VSLOTS), np.float32)
    wvT[:, live] = wvT_orig[:, src_ch[live]]

    dwv_orig = W_v_dw[:, 0].reshape(C, 9).astype(np.float32)
    dwv = np.zeros((VSLOTS, 9), np.float32)
    dwv[live] = dwv_orig[src_ch[live]]
    dwv_sc = np.stack([dwv[0:128], dwv[128:256]], 0).transpose(1, 0, 2).reshape(128, 18)

    diagv0 = np.zeros((128, 9, 128), np.float32)
    diagv1 = np.zeros((128, 9, 128), np.float32)
    for t in range(9):
        diagv0[np.arange(128), t, np.arange(128)] = dwv[0:128, t]
        diagv1[np.arange(128), t, np.arange(128)] = dwv[128:256, t]

    # qk conv weights DoubleRow-packed
    wqk8 = np.zeros((96, 2, O), np.float32)
    wqk8[:, 0, :] = wqkT[0:96, :]
    wqk8[:, 1, :] = wqkT[96:192, :]

    # fp8 diag depthwise matrices for the 3 qk slabs, DR slot layout
    _slot = {}
    for _pi, (_t0, _t1) in enumerate(DW_PAIRS):
        _slot[2 * _pi] = _t0
        _slot[2 * _pi + 1] = _t1
    _slot[8] = 4                                        # center tap; slot 9 stays zero
    diag8 = np.zeros((128, 3, 10, 128), np.float32)
    for _s in range(3):
        for _sl, _t in _slot.items():
            diag8[np.arange(128), _s, _sl, np.arange(128)] = dwqk[128 * _s : 128 * (_s + 1), _t]

    # proj weights per head: wphead[c, h, o] = WpT[48h+c, o]
    wprojT_orig = W_proj[:, :, 0, 0].T.astype(np.float32)  # [192 in, 192 out]
    wphead = np.zeros((48, 4, C), np.float32)
    for h in range(HEADS):
        wphead[:, h, :] = wprojT_orig[48 * h : 48 * (h + 1), :]

    gmask = np.zeros((96, 4 * 96), np.float32)
    for h in range(HEADS):
        gmask[np.arange(96), 96 * h + np.arange(96)] = 1.0

    temp = np.asarray(temperature).reshape(HEADS)
    tmmask = np.ones((96, 4), np.float32)
    tmmask[0:48, :] = temp[None, :]

    hsel = np.zeros((4, 4 * 48), np.float32)
    for h in range(HEADS):
        hsel[h, 48 * h : 48 * (h + 1)] = 1.0

    return {
        "wqk8": wqk8.reshape(96, 2 * O).astype(fp8),
        "wvT": wvT.astype(fp16),
        "wphead": wphead.reshape(48, 4 * C).astype(fp16),
        "diag8": diag8.reshape(128, 3 * 10 * 128).astype(fp8),
        "diagv0": diagv0.reshape(128, 9 * 128).astype(fp16),
        "diagv1": diagv1.reshape(128, 9 * 128).astype(fp16),
        "dwv_sc": dwv_sc.astype(np.float32),
        "gmask": gmask,
        "tmmask": tmmask,
        "idf32": np.eye(128, dtype=np.float32),
        "hsel": hsel,
    }


def kernel(x, W_qk, W_qk_dw, W_v, W_v_dw, W_proj, temperature):
    x = np.asarray(x, np.float32)
    b = x.shape[0]
    assert b == 8 and x.shape[1] == C

    if "nc" not in _CACHE:
        _CACHE["nc"] = build_nc()
    nc = _CACHE["nc"]

    static = _prep_static(
        np.asarray(W_qk), np.asarray(W_qk_dw), np.asarray(W_v),
        np.asarray(W_v_dw), np.asarray(W_proj), np.asarray(temperature),
    )
    in_maps = []
    for i in range(b):
        m = dict(static)
        xi = np.ascontiguousarray(x[i].reshape(C, HW))
        m["x"] = xi.astype(fp16)
        x8 = np.zeros((96, 2, HW), np.float32)
        x8[:, 0, :] = xi[0:96]
        x8[:, 1, :] = xi[96:192]
        m["x8"] = x8.reshape(96, 2 * HW).astype(fp8)
        in_maps.append(m)

    res = run_bass_kernel_spmd(nc, in_maps, core_ids=list(range(8)))
    y = np.stack([res.results[i]["y"].reshape(C, H, W) for i in range(8)])
    return y.astype(np.float32)


if __name__ == "__main__":
    t0 = time.time()
    nc = build_nc()
    print(f"build+compile: {time.time()-t0:.1f}s")
